# revision 1
# baseline (speedup 1.0000x reference)
"""Trainium2 Bass kernel for CausalSelfAttention (GQA + alibi, B=2, T=2048,
d_model=2048, 16 q heads / 4 kv heads).

Sharding: 8 cores = (batch b in {0,1}) x (kv-group g in {0..3}).
Each core computes, for its (b, g):
  - QKV^T slice:  [768, T]  (4 q heads pre-scaled by 1/sqrt(hd), 1 k head,
    1 v head) -- stage 1 runs in fb-pair passes so PE starts after the first
    weight chunk lands
  - causal attention for its 4 query heads (scores kept transposed:
    S^T[j, i] with keys j on partitions; alibi+mask applied as precomputed
    additive tiles on DVE; exp on ACT writes bf16 P; softmax denominator via
    a PE ones-matmul accumulation; normalization broadcast via a K=1 matmul)
  - partial output projection interleaved per 512-query block, bf16 output
Host sums the 4 partials per batch (upcast to f32) and adds proj_b.

qkv/proj matmuls run in bf16 (inputs pre-rounded on host); score matmuls in
float32r at >=256 free cols (diagonal tail blocks are left-extended to dodge
the narrow-f32r penalty); softmax arithmetic in fp32.
"""

import math

import numpy as np

D = 2048
T = 2048
NH = 16
KVH = 4
HD = 128
GRP = 4
B = 2
NCORE = 8
FB = 6          # qkv feature tiles of 128 (4 q heads + k + v)
NEG16 = -30000.0

_CACHE: dict = {}


# --------------------------------------------------------------------------
# device kernel
# --------------------------------------------------------------------------

def _build_nc():
    import concourse.mybir as mybir
    from concourse import bacc
    import concourse.tile as tile
    f32 = mybir.dt.float32
    f32r = mybir.dt.float32r
    bf16 = mybir.dt.bfloat16
    fp16 = mybir.dt.float16
    fp8 = mybir.dt.float8e4
    DR = mybir.MatmulPerfMode.DoubleRow
    Exp = mybir.ActivationFunctionType.Exp
    Ident = mybir.ActivationFunctionType.Identity
    add = mybir.AluOpType.add
    mult = mybir.AluOpType.mult

    nc = bacc.Bacc("TRN2", target_bir_lowering=False, debug=False,
                   num_devices=NCORE)

    # xt: [tb][dt][512] per-tb contiguous; wt: [fbp][dt][256] per-pass
    xt_d = nc.dram_tensor("xt", [128, 4 * 16 * 512], bf16,
                          kind="ExternalInput").ap()
    wt_d = nc.dram_tensor("wt", [128, 3 * 16 * 256], bf16,
                          kind="ExternalInput").ap()
    bq_d = nc.dram_tensor("bq", [128, FB], f32, kind="ExternalInput").ap()
    atr_d = nc.dram_tensor("atr", [128, 4 * 512], f32, kind="ExternalInput").ap()
    atd_d = nc.dram_tensor("atd", [128, 16 * 512], fp16,
                           kind="ExternalInput").ap()
    cb_d = nc.dram_tensor("cb", [128, 48], f32, kind="ExternalInput").ap()
    pt_d = nc.dram_tensor("pt", [128, 4 * T], bf16, kind="ExternalInput").ap()
    kn_d = nc.dram_tensor("kn", [128, 129], f32r, kind="ExternalInput").ap()
    knb_d = nc.dram_tensor("knb", [128, 129], bf16, kind="ExternalInput").ap()
    out_d = nc.dram_tensor("out", [T, D], bf16, kind="ExternalOutput").ap()

    with tile.TileContext(nc) as tc:
        with tc.tile_pool(name="persist", bufs=1) as pp, \
             tc.tile_pool(name="ps", bufs=2, space="PSUM") as ps_pool, \
             tc.tile_pool(name="s1p", bufs=2, space="PSUM") as s1p_pool, \
             tc.tile_pool(name="po", bufs=2, space="PSUM") as po_pool, \
             tc.tile_pool(name="dr", bufs=2, space="PSUM") as dr_pool:

            qkvT = pp.tile([128, 5 * T], f32r, name="qkvT", tag="qkvT")
            vTb = pp.tile([128, T], bf16, name="vTb", tag="vTb")
            v_all = pp.tile([128, T], bf16, name="v_all", tag="v_all")
            oT = pp.tile([128, 4 * T], bf16, name="oT", tag="oT")
            bq = pp.tile([128, FB], f32, name="bqs", tag="bqs")
            cb = pp.tile([128, 48], f32, name="cbs", tag="cbs")
            ones = pp.tile([1, 128], f32r, name="ones1", tag="ones1")
            onesb = pp.tile([128, 1], bf16, name="onesb", tag="onesb")
            identb = pp.tile([128, 128], bf16, name="identb", tag="identb")
            wt = pp.tile([128, 3 * 4096], bf16, name="wt", tag="wt")
            atr = pp.tile([128, 4 * 512], f32, name="atr", tag="atr")
            atd = pp.tile([128, 16 * 512], fp16, name="atd", tag="atd")
            pt = pp.tile([128, 4 * T], bf16, name="pt", tag="pt")
            kT = qkvT[:, 4 * T:5 * T]

            with tc.tile_pool(name="s1x", bufs=2) as s1x, \
                 tc.tile_pool(name="s2w", bufs=2) as s2w, \
                 tc.tile_pool(name="s3o", bufs=2) as s3o:

                xts = [None] * 4
                xts[0] = s1x.tile([128, 16 * 512], bf16, name="xt", tag="xt")
                xts[1] = s1x.tile([128, 16 * 512], bf16, name="xt", tag="xt")

                # ---- DMA issue order: minimal working set first ----------
                nc.sync.dma_start(wt[:, 0:2048], wt_d[:, 0:2048])
                nc.sync.dma_start(xts[0][:, 0:2048], xt_d[:, 0:2048])
                nc.sync.dma_start(wt[:, 2048:4096], wt_d[:, 2048:4096])
                nc.sync.dma_start(xts[0][:, 2048:8192], xt_d[:, 2048:8192])
                nc.sync.dma_start(wt[:, 4096:8192], wt_d[:, 4096:8192])
                nc.sync.dma_start(wt[:, 8192:12288], wt_d[:, 8192:12288])
                nc.sync.dma_start(bq, bq_d)
                nc.sync.dma_start(cb, cb_d)
                nc.sync.dma_start(ones, kn_d[0:1, 1:129])
                nc.sync.dma_start(onesb, knb_d[:, 0:1])
                nc.sync.dma_start(identb, knb_d[:, 1:129])
                nc.sync.dma_start(xts[1], xt_d[:, 8192:16384])
                nc.sync.dma_start(atr, atr_d)
                nc.sync.dma_start(atd, atd_d)
                nc.sync.dma_start(pt, pt_d)

                for tb in range(4):
                    xt = xts[tb]
                    if tb + 2 < 4:
                        xts[tb + 2] = s1x.tile([128, 16 * 512], bf16,
                                               name="xt", tag="xt")
                        nc.sync.dma_start(
                            xts[tb + 2],
                            xt_d[:, (tb + 2) * 8192:(tb + 3) * 8192])
                    # ---- stage 1: fb-pair passes over 16 dt chunks -------
                    for p in range(3):
                        accs = [s1p_pool.tile([128, 512], f32, name="acc",
                                              tag="s1p") for _ in range(2)]
                        for dt_ in range(16):
                            for k in range(2):
                                nc.tensor.matmul(
                                    accs[k],
                                    wt[:, p * 4096 + dt_ * 256 + k * 128:
                                            p * 4096 + dt_ * 256 + k * 128 + 128],
                                    xt[:, dt_ * 512:(dt_ + 1) * 512],
                                    start=(dt_ == 0), stop=(dt_ == 15))
                        for k in range(2):
                            fb = 2 * p + k
                            if fb < 5:
                                dst = qkvT[:, fb * T + tb * 512:
                                               fb * T + tb * 512 + 512]
                            else:
                                dst = vTb[:, tb * 512:tb * 512 + 512]
                            nc.scalar.activation(dst, accs[k], Ident,
                                                 bias=bq[:, fb:fb + 1],
                                                 scale=1.0)

                    # ---- V transposes for this tb's four 128-blocks ------
                    # V stored as fp8 pair (v8 + vlo) for DoubleRow matmuls
                    for jt in range(4 * tb, 4 * tb + 4):
                        pv = ps_pool.tile([128, 128], bf16, name="pv",
                                          tag="ps")
                        nc.tensor.transpose(
                            pv, vTb[:, jt * 128:(jt + 1) * 128], identb)
                        nc.vector.tensor_copy(
                            v_all[:, jt * 128:(jt + 1) * 128], pv)

                    # ---- proj jobs for tb-1, woven into attention below --
                    proj_jobs = []
                    if tb >= 1:
                        for t128 in range(4 * (tb - 1), 4 * (tb - 1) + 4):
                            for ob in range(4):
                                proj_jobs.append((t128, ob))
                    ostages = {}

                    def emit_proj(job):
                        t128, ob = job
                        if t128 not in ostages:
                            ostages[t128] = s3o.tile([128, 2048], bf16,
                                                     name="ostage",
                                                     tag="ostage")
                        ostage = ostages[t128]
                        acc2 = s1p_pool.tile([128, 512], f32, name="acc2",
                                             tag="s1p")
                        for dt_ in range(4):
                            nc.tensor.matmul(
                                acc2,
                                oT[:, dt_ * T + t128 * 128:
                                        dt_ * T + t128 * 128 + 128],
                                pt[:, dt_ * T + ob * 512:
                                        dt_ * T + ob * 512 + 512],
                                start=(dt_ == 0), stop=(dt_ == 3))
                        if ob < 2:
                            nc.scalar.copy(
                                ostage[:, ob * 512:(ob + 1) * 512], acc2)
                        else:
                            nc.vector.tensor_copy(
                                ostage[:, ob * 512:(ob + 1) * 512], acc2)
                        if ob == 3:
                            nc.sync.dma_start(
                                out_d[t128 * 128:(t128 + 1) * 128, :],
                                ostage)

                    # ---- attention at ib = tb, head pairs interleaved ----
                    ib = tb
                    njb = 4 * (ib + 1)
                    iters_left = [4 * njb]
                    for hp in (0, 2):
                        chains = []
                        for h in (hp, hp + 1):
                            opsum = po_pool.tile([128, 512], f32,
                                                 name="opsum", tag="po")
                            dred = dr_pool.tile([1, 512], f32, name="dred",
                                                tag="dr")
                            chains.append((h, opsum, dred))

                        def flush(ent):
                            (h, opsum, dred, jb, c0, psb) = ent
                            nc.tensor.matmul(
                                dred[:, c0:512], onesb, psb[:, c0:512],
                                start=(jb == 0), stop=(jb == njb - 1),
                                skip_group_check=True)
                            nc.tensor.matmul(
                                opsum[:, c0:512],
                                v_all[:, jb * 128:(jb + 1) * 128],
                                psb[:, c0:512],
                                start=(jb == 0), stop=(jb == njb - 1),
                                skip_group_check=True)
                            if jb == njb - 1:
                                rsum = s2w.tile([1, 512], f32r,
                                                name="rsum", tag="rsum")
                                with nc.allow_low_precision(
                                        reason="softmax recip in f32r"):
                                    nc.vector.reciprocal(rsum, dred)
                                rps = ps_pool.tile([128, 512], f32,
                                                   name="rps", tag="ps")
                                nc.tensor.matmul(rps, ones, rsum,
                                                 start=True, stop=True)
                                rsb = s2w.tile([128, 512], f32,
                                               name="rsb", tag="rsb")
                                nc.scalar.copy(rsb, rps)
                                nc.vector.tensor_tensor(
                                    oT[:, h * T + ib * 512:
                                          h * T + ib * 512 + 512],
                                    opsum, rsb, mult)

                        pend = []
                        for jb in range(njb):
                          for (h, opsum, dred) in chains:
                            qT = qkvT[:, h * T:(h + 1) * T]
                            i0 = ib * 512
                            dd = jb - 4 * ib
                            c0 = 128 * dd if dd > 0 else 0
                            m0 = min(c0, 256)
                            spsum = ps_pool.tile([128, 512], f32,
                                                 name="spsum", tag="ps")
                            nc.tensor.matmul(
                                spsum[:, m0:512],
                                kT[:, jb * 128:(jb + 1) * 128],
                                qT[:, i0 + m0:i0 + 512],
                                start=True, stop=True)
                            ssb = s2w.tile([128, 512], f32, name="ssb",
                                           tag="ssb", bufs=6)
                            if dd >= 0:   # diagonal band (masked fp16)
                                nc.vector.tensor_tensor(
                                    ssb[:, c0:512], spsum[:, c0:512],
                                    atd[:, (h * 4 + dd) * 512 + c0:
                                          (h * 4 + dd + 1) * 512], add)
                                bias = 0.0
                            else:         # strictly-lower blocks
                                nc.vector.tensor_tensor(
                                    ssb, spsum,
                                    atr[:, h * 512:(h + 1) * 512], add)
                                k_ = 4 * ib - jb
                                bias = cb[:, h * 12 + k_ - 1: h * 12 + k_]
                            psb = s2w.tile([128, 512], bf16, name="psb",
                                           tag="psb", bufs=8)
                            nc.scalar.activation(psb[:, c0:512],
                                                 ssb[:, c0:512],
                                                 Exp, bias=bias, scale=1.0)
                            pend.append((h, opsum, dred, jb, c0, psb))
                            if len(pend) > 3:
                                flush(pend.pop(0))
                            # weave in proj chains for tb-1 to keep PE dense
                            avail = len(proj_jobs) - 2
                            nj = -(-avail // iters_left[0]) if avail > 0 else 0
                            for _ in range(nj):
                                emit_proj(proj_jobs.pop(0))
                            iters_left[0] -= 1
                        for ent in pend:
                            flush(ent)
                            if proj_jobs:
                                emit_proj(proj_jobs.pop(0))
                    while proj_jobs:
                        emit_proj(proj_jobs.pop(0))

                    if tb == 3:
                        # ---- final proj for the last query block ---------
                        proj_jobs = [(t128, ob)
                                     for t128 in range(12, 16)
                                     for ob in range(4)]
                        ostages = {}
                        while proj_jobs:
                            emit_proj(proj_jobs.pop(0))

    nc.compile()
    return nc


def get_nc():
    if "nc" not in _CACHE:
        _CACHE["nc"] = _build_nc()
    return _CACHE["nc"]


# --------------------------------------------------------------------------
# host-side packing
# --------------------------------------------------------------------------

def _expected_slopes():
    return 2.0 ** (-8.0 * (np.arange(1, NH + 1) / NH))  # float64


def _check_structure(attn_mask, alibi_bias):
    """Return exact float64 alibi slopes if inputs match the expected
    causal-mask + rank-1 alibi structure, else None."""
    am = np.asarray(attn_mask)
    if am.shape != (1, 1, T, T):
        return None
    if not np.array_equal(am[0, 0], np.tril(np.ones((T, T), dtype=bool))):
        return None
    al = np.asarray(alibi_bias, dtype=np.float32)
    if al.shape != (1, NH, T, T):
        return None
    slopes = _expected_slopes()
    if not np.allclose(al[0, :, 0, 1], slopes.astype(np.float32),
                       rtol=1e-6, atol=1e-8):
        return None
    idx = np.arange(T, dtype=np.float64)
    rel = idx[None, :] - idx[:, None]
    for h in range(NH):
        ref = (slopes[h] * rel).astype(np.float32)
        if not np.array_equal(al[0, h], ref):
            if not np.allclose(al[0, h], ref, rtol=1e-5, atol=1e-4):
                return None
    return slopes


def _pack_core_inputs(x, qkv_w, qkv_b, proj_w, slopes):
    import ml_dtypes
    bf = ml_dtypes.bfloat16
    x = np.asarray(x, dtype=np.float32)
    qkv_w = np.asarray(qkv_w, dtype=np.float32)
    qkv_b = np.asarray(qkv_b, dtype=np.float32)
    proj_w = np.asarray(proj_w, dtype=np.float32)
    inv = np.float32(1.0 / math.sqrt(HD))

    xts = []
    for b in range(B):
        # [128, dt, T] -> regroup to [128, tb, dt, 512]
        xt = (x[b].T.reshape(16, 128, T).transpose(1, 0, 2)
              .reshape(128, 16, 4, 512).transpose(0, 2, 1, 3)
              .reshape(128, 4 * 16 * 512))
        xts.append(np.ascontiguousarray(xt.astype(bf)))

    per_g = []
    jj = np.arange(128, dtype=np.float64)[:, None]
    ii = np.arange(512, dtype=np.float64)[None, :]
    for g in range(KVH):
        Wq = qkv_w[512 * g:512 * (g + 1)] * inv
        Wk = qkv_w[D + 128 * g: D + 128 * (g + 1)]
        Wv = qkv_w[D + 512 + 128 * g: D + 512 + 128 * (g + 1)]
        Wc = np.concatenate([Wq, Wk, Wv], axis=0)          # [768, 2048]
        # [128, dt, fb, 128] -> [128, fbp(3), dt, 256]
        wt = (Wc.T.reshape(16, 128, 768).transpose(1, 0, 2)
              .reshape(128, 16, 3, 256).transpose(0, 2, 1, 3)
              .reshape(128, 3 * 16 * 256))
        wt = np.ascontiguousarray(wt.astype(bf))
        bc = np.concatenate([qkv_b[512 * g:512 * (g + 1)] * inv,
                             qkv_b[D + 128 * g: D + 128 * (g + 1)],
                             qkv_b[D + 512 + 128 * g: D + 512 + 128 * (g + 1)]])
        bqp = np.ascontiguousarray(bc.reshape(FB, 128).T)  # [128, 6]

        atr = np.empty((128, 4 * 512), dtype=np.float32)
        atd = np.empty((128, 16 * 512), dtype=np.float16)
        cbp = np.empty((128, 48), dtype=np.float32)
        for h in range(GRP):
            s = slopes[4 * g + h]
            atr[:, h * 512:(h + 1) * 512] = (s * (jj - ii)).astype(np.float32)
            for dd in range(4):
                A = (s * (jj - ii + 128 * dd)).astype(np.float16)
                A[(jj + 128 * dd - ii) > 0] = np.float16(NEG16)
                atd[:, (h * 4 + dd) * 512:(h * 4 + dd + 1) * 512] = A
            for k_ in range(1, 13):
                cbp[:, h * 12 + k_ - 1] = np.float32(s * (-128.0 * k_))

        ptp = np.ascontiguousarray(
            proj_w[:, 512 * g:512 * (g + 1)].T
            .reshape(4, 128, T).transpose(1, 0, 2).reshape(128, 4 * T)
            .astype(bf))
        per_g.append({"wt": wt, "bq": bqp, "atr": atr, "atd": atd,
                      "cb": cbp, "pt": ptp})

    kn = np.zeros((128, 129), dtype=np.float32)
    kn[0, 1:129] = 1.0                  # ones row     [1, 128]
    knb = np.zeros((128, 129), dtype=ml_dtypes.bfloat16)
    knb[:, 0] = 1.0                     # ones column  [128, 1]
    knb[:, 1:129] = np.eye(128, dtype=np.float32)

    in_maps = []
    for c in range(NCORE):
        b, g = divmod(c, KVH)
        m = dict(per_g[g])
        m["xt"] = xts[b]
        m["kn"] = kn
        m["knb"] = knb
        in_maps.append(m)
    return in_maps


# --------------------------------------------------------------------------
# numpy fallback (only used if inputs don't match the expected structure)
# --------------------------------------------------------------------------

def _numpy_reference(x, attn_mask, alibi_bias, qkv_w, qkv_b, proj_w, proj_b):
    x = np.asarray(x, dtype=np.float32)
    b, t, c = x.shape
    qkv = x @ qkv_w.T + qkv_b
    q = qkv[..., :D].reshape(b, t, KVH, GRP, HD).transpose(0, 2, 3, 1, 4)
    k = qkv[..., D:D + 512].reshape(b, t, KVH, HD).transpose(0, 2, 1, 3)
    v = qkv[..., D + 512:].reshape(b, t, KVH, HD).transpose(0, 2, 1, 3)
    scale = 1.0 / math.sqrt(HD)
    att = np.einsum("bkgtd,bksd->bkgts", q, k).astype(np.float32) * scale
    att = att + np.asarray(alibi_bias).reshape(1, KVH, GRP, t, t)
    mask = np.asarray(attn_mask)[:, :, None]
    att = np.where(mask, att, -np.inf)
    att = att - att.max(axis=-1, keepdims=True)
    np.exp(att, out=att)
    att /= att.sum(axis=-1, keepdims=True)
    out = np.einsum("bkgts,bksd->bkgtd", att, v)
    out = out.transpose(0, 3, 1, 2, 4).reshape(b, t, c)
    return (out @ proj_w.T + proj_b).astype(np.float32)


# --------------------------------------------------------------------------
# entry point
# --------------------------------------------------------------------------

def kernel(x, attn_mask, alibi_bias, qkv_w, qkv_b, proj_w, proj_b):
    from concourse import bass_utils

    slopes = _check_structure(attn_mask, alibi_bias)
    if slopes is None:
        return _numpy_reference(x, attn_mask, alibi_bias, qkv_w, qkv_b,
                                proj_w, proj_b)

    nc = get_nc()
    in_maps = _pack_core_inputs(x, qkv_w, qkv_b, proj_w, slopes)
    res = bass_utils.run_bass_kernel_spmd(nc, in_maps,
                                          core_ids=list(range(NCORE)))
    proj_b = np.asarray(proj_b, dtype=np.float32)
    out = np.empty((B, T, D), dtype=np.float32)
    for b in range(B):
        acc = res.results[4 * b + 0]["out"].astype(np.float32)
        for g in range(1, KVH):
            acc = acc + res.results[4 * b + g]["out"].astype(np.float32)
        out[b] = acc + proj_b
    return out



# revision 34
# speedup vs baseline: 1.1731x; 1.1731x over previous
"""Trainium2 Bass kernel for CausalSelfAttention (GQA + alibi, B=2, T=2048,
d_model=2048, 16 q heads / 4 kv heads).

Sharding: 8 cores = (batch b in {0,1}) x (kv-group g in {0..3}).
Each core computes, for its (b, g):
  - QKV^T slice [768, T] via 3-term fp8 DoubleRow matmuls
    (w8*x8 + w8*xlo + wlo*x8; weights host-split hi/lo with per-fb scales,
    x host-split hi/lo) -- ~bf16 accuracy at 2x the bf16 MAC rate
  - causal attention for its 4 query heads (scores transposed S^T[j, i],
    f32r matmuls; alibi+mask additive tiles on DVE; exp on ACT -> bf16 P;
    softmax denominator via near-free transposed tiny matmuls
    (lhsT=P chunk, rhs=ones column -> [128,1] outputs); reciprocal in fp16;
    per-column broadcast rebuilt via 4 single-column PE transposes + one
    Pool-engine partition_broadcast)
  - normalized attention output written as fp8 hi+lo pair (o8 + olo)
  - partial output projection via 3-term fp8 DoubleRow matmuls
    (o8*pt8 + olo*pt8 + o8*ptlo), interleaved per 512-query block
Host sums the 4 partials per batch (upcast to f32) and adds proj_b.
"""

import math

import numpy as np

D = 2048
T = 2048
NH = 16
KVH = 4
HD = 128
GRP = 4
B = 2
NCORE = 8
FB = 6          # qkv feature tiles of 128 (4 q heads + k + v)
NEG16 = -30000.0
SQ = 256.0      # fp8 scale for q weight rows (pre-scaled by 1/sqrt(hd))
SKV = 32.0      # fp8 scale for k/v weight rows
SPW = 32.0      # fp8 scale for proj weights

_CACHE: dict = {}


# --------------------------------------------------------------------------
# device kernel
# --------------------------------------------------------------------------

def _build_nc():
    import concourse.mybir as mybir
    from concourse import bacc
    import concourse.tile as tile
    f32 = mybir.dt.float32
    f32r = mybir.dt.float32r
    bf16 = mybir.dt.bfloat16
    fp16 = mybir.dt.float16
    fp8 = mybir.dt.float8e4
    DR = mybir.MatmulPerfMode.DoubleRow
    Exp = mybir.ActivationFunctionType.Exp
    Ident = mybir.ActivationFunctionType.Identity
    add = mybir.AluOpType.add
    mult = mybir.AluOpType.mult
    subtract = mybir.AluOpType.subtract

    nc = bacc.Bacc("TRN2", target_bir_lowering=False, debug=False,
                   num_devices=NCORE)

    # x8/xlo: [tb][dt][512] per-tb contiguous; w8/wlo: [fb][dt][128]
    x8_d = nc.dram_tensor("x8", [128, 4 * 16 * 512], fp8,
                          kind="ExternalInput").ap()
    xlo_d = nc.dram_tensor("xlo", [128, 4 * 16 * 512], fp8,
                           kind="ExternalInput").ap()
    w8_d = nc.dram_tensor("w8", [128, 6 * 16 * 128], fp8,
                          kind="ExternalInput").ap()
    wlo_d = nc.dram_tensor("wlo", [128, 6 * 16 * 128], fp8,
                           kind="ExternalInput").ap()
    bq_d = nc.dram_tensor("bq", [128, FB], f32, kind="ExternalInput").ap()
    atr_d = nc.dram_tensor("atr", [128, 4 * 512], f32,
                           kind="ExternalInput").ap()
    atd_d = nc.dram_tensor("atd", [128, 16 * 512], fp16,
                           kind="ExternalInput").ap()
    cb_d = nc.dram_tensor("cb", [128, 48], f32, kind="ExternalInput").ap()
    abt_d = nc.dram_tensor("abt", [2, 4 * 640], f32r,
                           kind="ExternalInput").ap()
    pt8_d = nc.dram_tensor("pt8", [128, 4 * T], fp8, kind="ExternalInput").ap()
    ptlo_d = nc.dram_tensor("ptlo", [128, 4 * T], fp8,
                            kind="ExternalInput").ap()
    knb_d = nc.dram_tensor("knb", [128, 129], bf16, kind="ExternalInput").ap()
    knh_d = nc.dram_tensor("knh", [128, 128], f32r, kind="ExternalInput").ap()
    onh_d = nc.dram_tensor("onh", [1, 128], fp16, kind="ExternalInput").ap()
    out_d = nc.dram_tensor("out", [T, D], bf16, kind="ExternalOutput").ap()
    import os
    _dbg = os.environ.get("KDBG") == "1"
    if _dbg:
        dbgq_d = nc.dram_tensor("dbgq", [128, 5 * T], f32,
                                kind="ExternalOutput").ap()
        dbgv_d = nc.dram_tensor("dbgv", [128, T], bf16,
                                kind="ExternalOutput").ap()
        dbgo_d = nc.dram_tensor("dbgo", [128, 4 * T], mybir.dt.uint8,
                                kind="ExternalOutput").ap()
        dbgl_d = nc.dram_tensor("dbgl", [128, 4 * T], mybir.dt.uint8,
                                kind="ExternalOutput").ap()

    with tile.TileContext(nc) as tc:
        with tc.tile_pool(name="persist", bufs=1) as pp, \
             tc.tile_pool(name="ps", bufs=2, space="PSUM") as ps_pool, \
             tc.tile_pool(name="s1p", bufs=2, space="PSUM") as s1p_pool, \
             tc.tile_pool(name="po", bufs=2, space="PSUM") as po_pool, \
             tc.tile_pool(name="dr", bufs=2, space="PSUM") as dr_pool:

            qkvT = pp.tile([128, 5 * T], f32r, name="qkvT", tag="qkvT")
            vTb = pp.tile([128, T], bf16, name="vTb", tag="vTb")
            v_all = pp.tile([128, T], bf16, name="v_all", tag="v_all")
            o8 = pp.tile([128, 4 * T], fp8, name="o8", tag="o8")
            olo = pp.tile([128, 4 * T], fp8, name="olo", tag="olo")
            bq = pp.tile([128, FB], f32, name="bqs", tag="bqs")
            cb = pp.tile([128, 48], f32, name="cbs", tag="cbs")
            onesb = pp.tile([128, 1], bf16, name="onesb", tag="onesb")
            identb = pp.tile([128, 128], bf16, name="identb", tag="identb")
            identr = pp.tile([128, 128], f32r, name="identr", tag="identr")
            onesh = pp.tile([1, 128], fp16, name="onesh", tag="onesh")
            w8 = pp.tile([128, 6 * 2048], fp8, name="w8", tag="w8")
            wlo = pp.tile([128, 6 * 2048], fp8, name="wlo", tag="wlo")
            abt = pp.tile([2, 4 * 640], f32r, name="abt", tag="abt")
            atr = pp.tile([128, 4 * 512], f32, name="atr", tag="atr")
            atd = pp.tile([128, 16 * 512], fp16, name="atd", tag="atd")
            pt8 = pp.tile([128, 4 * T], fp8, name="pt8", tag="pt8")
            ptlo = pp.tile([128, 4 * T], fp8, name="ptlo", tag="ptlo")
            kT = qkvT[:, 4 * T:5 * T]
            o8v = o8.rearrange("p (h t) -> p h t", h=4)
            olov = olo.rearrange("p (h t) -> p h t", h=4)
            pt8v = pt8.rearrange("p (h t) -> p h t", h=4)
            ptlov = ptlo.rearrange("p (h t) -> p h t", h=4)

            with tc.tile_pool(name="s1x", bufs=2) as s1x, \
                 tc.tile_pool(name="s1xl", bufs=2) as s1xl, \
                 tc.tile_pool(name="s2w", bufs=2) as s2w, \
                 tc.tile_pool(name="s3o", bufs=2) as s3o:

                x8s = [None] * 4
                xlos = [None] * 4
                x8s[0] = s1x.tile([128, 16 * 512], fp8, name="x8t", tag="x8t")
                x8s[1] = s1x.tile([128, 16 * 512], fp8, name="x8t", tag="x8t")
                xlos[0] = s1xl.tile([128, 16 * 512], fp8, name="xlt", tag="xlt")
                xlos[1] = s1xl.tile([128, 16 * 512], fp8, name="xlt", tag="xlt")

                # ---- DMA issue order: minimal working set first ----------
                nc.sync.dma_start(w8[:, 0:2048], w8_d[:, 0:2048])
                nc.sync.dma_start(x8s[0][:, 0:4096], x8_d[:, 0:4096])
                nc.sync.dma_start(x8s[0][:, 4096:8192], x8_d[:, 4096:8192])
                nc.sync.dma_start(wlo[:, 0:2048], wlo_d[:, 0:2048])
                nc.sync.dma_start(bq, bq_d)
                nc.sync.dma_start(xlos[0][:, 0:4096], xlo_d[:, 0:4096])
                nc.sync.dma_start(xlos[0][:, 4096:8192], xlo_d[:, 4096:8192])
                for fb_ in range(1, 6):
                    lo = fb_ * 2048
                    nc.sync.dma_start(w8[:, lo:lo + 2048],
                                      w8_d[:, lo:lo + 2048])
                    nc.sync.dma_start(wlo[:, lo:lo + 2048],
                                      wlo_d[:, lo:lo + 2048])
                nc.sync.dma_start(cb, cb_d)
                nc.sync.dma_start(abt, abt_d)
                nc.sync.dma_start(onesb, knb_d[:, 0:1])
                nc.sync.dma_start(identb, knb_d[:, 1:129])
                nc.sync.dma_start(identr, knh_d)
                nc.sync.dma_start(onesh, onh_d)
                nc.sync.dma_start(atr, atr_d)
                nc.sync.dma_start(x8s[1], x8_d[:, 8192:16384])
                nc.sync.dma_start(xlos[1], xlo_d[:, 8192:16384])
                nc.sync.dma_start(atd, atd_d)
                nc.sync.dma_start(pt8, pt8_d)
                nc.sync.dma_start(ptlo, ptlo_d)

                tails = []  # deferred per-(h,ib) softmax-normalization tails

                def drain_tail():
                    if tails:
                        tails.pop(0)()

                for tb in range(4):
                    x8t, xlot = x8s[tb], xlos[tb]
                    if tb + 2 < 4:
                        x8s[tb + 2] = s1x.tile([128, 16 * 512], fp8,
                                               name="x8t", tag="x8t")
                        xlos[tb + 2] = s1xl.tile([128, 16 * 512], fp8,
                                                 name="xlt", tag="xlt")
                        nc.sync.dma_start(
                            x8s[tb + 2],
                            x8_d[:, (tb + 2) * 8192:(tb + 3) * 8192])
                        nc.sync.dma_start(
                            xlos[tb + 2],
                            xlo_d[:, (tb + 2) * 8192:(tb + 3) * 8192])
                    x8v = x8t.rearrange("p (d n) -> p d n", d=16)
                    xlov = xlot.rearrange("p (d n) -> p d n", d=16)
                    w8v = w8.rearrange("p (f d m) -> p f d m", f=6, d=16)
                    wlov = wlo.rearrange("p (f d m) -> p f d m", f=6, d=16)

                    # ---- stage 1: per-fb 3-term fp8 DoubleRow chains -----
                    for fb in range(FB):
                        acc = s1p_pool.tile([128, 512], f32, name="acc",
                                            tag="s1p")
                        nmm = 0
                        for wv, xv in ((w8v, x8v), (wlov, x8v), (w8v, xlov)):
                            for dp in range(8):
                                nc.tensor.matmul(
                                    acc,
                                    wv[:, fb, 2 * dp:2 * dp + 2, :],
                                    xv[:, 2 * dp:2 * dp + 2, :],
                                    start=(nmm == 0), stop=(nmm == 23),
                                    perf_mode=DR)
                                nmm += 1
                        if fb < 4:
                            dst = qkvT[:, fb * T + tb * 512:
                                           fb * T + tb * 512 + 512]
                            sc = 1.0 / SQ
                        elif fb == 4:
                            dst = kT[:, tb * 512:tb * 512 + 512]
                            sc = 1.0 / SKV
                        else:
                            dst = vTb[:, tb * 512:tb * 512 + 512]
                            sc = 1.0 / SKV
                        nc.scalar.activation(dst, acc, Ident,
                                             bias=bq[:, fb:fb + 1],
                                             scale=sc)
                        if fb in (2, 4):
                            drain_tail()

                    # ---- V transposes for this tb's four 128-blocks ------
                    for jt in range(4 * tb, 4 * tb + 4):
                        pv = ps_pool.tile([128, 128], bf16, name="pv",
                                          tag="ps")
                        nc.tensor.transpose(
                            pv, vTb[:, jt * 128:(jt + 1) * 128], identb)
                        nc.vector.tensor_copy(
                            v_all[:, jt * 128:(jt + 1) * 128], pv)

                    # ---- proj jobs for tb-1, woven into attention below --
                    proj_jobs = []
                    if tb >= 1:
                        for t128 in range(4 * (tb - 1), 4 * (tb - 1) + 4):
                            for ob in range(4):
                                proj_jobs.append((t128, ob))
                    ostages = {}

                    def emit_proj(job):
                        t128, ob = job
                        if t128 not in ostages:
                            ostages[t128] = s3o.tile([128, 2048], bf16,
                                                     name="ostage",
                                                     tag="ostage")
                        ostage = ostages[t128]
                        acc2 = s1p_pool.tile([128, 512], f32, name="acc2",
                                             tag="s1p")
                        nmm = 0
                        for hp_ in range(2):
                            for ov, pv_ in ((o8v, pt8v), (olov, pt8v),
                                            (o8v, ptlov)):
                                nc.tensor.matmul(
                                    acc2,
                                    ov[:, 2 * hp_:2 * hp_ + 2,
                                       t128 * 128:t128 * 128 + 128],
                                    pv_[:, 2 * hp_:2 * hp_ + 2,
                                        ob * 512:ob * 512 + 512],
                                    start=(nmm == 0), stop=(nmm == 5),
                                    perf_mode=DR)
                                nmm += 1
                        if ob < 2:
                            nc.scalar.activation(
                                ostage[:, ob * 512:(ob + 1) * 512], acc2,
                                Ident, scale=1.0 / SPW)
                        else:
                            nc.vector.tensor_scalar_mul(
                                ostage[:, ob * 512:(ob + 1) * 512], acc2,
                                1.0 / SPW)
                        if ob == 1:
                            nc.sync.dma_start(
                                out_d[t128 * 128:(t128 + 1) * 128, 0:1024],
                                ostage[:, 0:1024])
                        elif ob == 3:
                            nc.sync.dma_start(
                                out_d[t128 * 128:(t128 + 1) * 128,
                                      1024:2048],
                                ostage[:, 1024:2048])

                    # ---- attention at ib = tb, head pairs interleaved ----
                    ib = tb
                    njb = 4 * (ib + 1)
                    iters_left = [4 * njb]
                    for hp in (0, 2):
                        chains = []
                        for idx, h in enumerate((hp, hp + 1)):
                            opsum = po_pool.tile([128, 512], f32,
                                                 name="opsum", tag="po")
                            den = dr_pool.tile([128, 4], f32, name="den",
                                               tag="dr")
                            chains.append((h, opsum, den))

                        def flush(ent):
                            (h, opsum, den, jb, c0, psb) = ent
                            for c in range(c0 // 128, 4):
                                nc.tensor.matmul(
                                    den[:, c:c + 1],
                                    psb[:, c * 128:(c + 1) * 128],
                                    onesb,
                                    start=(jb == 0 and c == 0),
                                    stop=(jb == 4 * ib + c),
                                    skip_group_check=True)
                            nc.tensor.matmul(
                                opsum[:, c0:512],
                                v_all[:, jb * 128:(jb + 1) * 128],
                                psb[:, c0:512],
                                start=(jb == 0), stop=(jb == njb - 1),
                                skip_group_check=True)
                            if jb == njb - 1:
                                rden = s2w.tile([128, 4], f32r,
                                                name="rden", tag="rden",
                                                bufs=4)
                                with nc.allow_low_precision(
                                        reason="softmax recip in f32r"):
                                    nc.vector.reciprocal(rden, den)
                                # snapshot opsum so the PSUM bank frees now
                                osb = s2w.tile([128, 512], fp16,
                                               name="osb", tag="osb",
                                               bufs=4)
                                nc.vector.tensor_copy(osb, opsum)

                                def tail(h=h, ib=ib, rden=rden, osb=osb):
                                    rps = ps_pool.tile([128, 512], f32,
                                                       name="rps", tag="ps")
                                    rrow = rps[0:1, 0:512].bitcast(f32r)
                                    for c in range(4):
                                        nc.tensor.matmul(
                                            rrow[:, c * 128:(c + 1) * 128],
                                            rden[:, c:c + 1], identr,
                                            is_transpose=True,
                                            start=True, stop=True,
                                            skip_group_check=True)
                                    rsum = s2w.tile([1, 512], fp16,
                                                    name="rsum", tag="rsum")
                                    nc.scalar.copy(rsum, rps[0:1, 0:512])
                                    nc.tensor.matmul(rps, onesh, rsum,
                                                     start=True, stop=True,
                                                     skip_group_check=True)
                                    tmp = s2w.tile([128, 512], f32,
                                                   name="otmp", tag="otmp")
                                    nc.vector.tensor_tensor(tmp, osb, rps,
                                                            mult)
                                    od = o8v[:, h, ib * 512:ib * 512 + 512]
                                    ol = olov[:, h, ib * 512:ib * 512 + 512]
                                    nc.vector.tensor_copy(od, tmp)
                                    nc.vector.tensor_tensor(
                                        ol, tmp, od, subtract)
                                tails.append(tail)

                        pend = []
                        for jb in range(njb):
                          for (h, opsum, den) in chains:
                            qT = qkvT[:, h * T:(h + 1) * T]
                            i0 = ib * 512
                            dd = jb - 4 * ib
                            c0 = 128 * dd if dd > 0 else 0
                            m0 = min(c0, 256)
                            spsum = ps_pool.tile([128, 512], f32,
                                                 name="spsum", tag="ps")
                            psb = s2w.tile([128, 512], bf16, name="psb",
                                           tag="psb", bufs=8)
                            if dd >= 0:   # diagonal band (masked fp16)
                                nc.tensor.matmul(
                                    spsum[:, m0:512],
                                    kT[:, jb * 128:(jb + 1) * 128],
                                    qT[:, i0 + m0:i0 + 512],
                                    start=True, stop=True)
                                ssb = s2w.tile([128, 512], fp16, name="ssb",
                                               tag="ssb", bufs=4)
                                nc.vector.tensor_tensor(
                                    ssb[:, c0:512], spsum[:, c0:512],
                                    atd[:, (h * 4 + dd) * 512 + c0:
                                          (h * 4 + dd + 1) * 512], add)
                                nc.scalar.activation(psb[:, c0:512],
                                                     ssb[:, c0:512],
                                                     Exp, bias=0.0, scale=1.0)
                            elif jb % 2 == 0:  # lower: rank-2 alibi on PE
                                nc.tensor.matmul(
                                    spsum,
                                    kT[:, jb * 128:(jb + 1) * 128],
                                    qT[:, i0:i0 + 512],
                                    start=True, stop=False,
                                    skip_group_check=True)
                                nc.tensor.matmul(
                                    spsum,
                                    abt[:, h * 640:h * 640 + 128],
                                    abt[:, h * 640 + 128:(h + 1) * 640],
                                    start=False, stop=True,
                                    skip_group_check=True)
                                k_ = 4 * ib - jb
                                bias = cb[:, h * 12 + k_ - 1: h * 12 + k_]
                                nc.scalar.activation(psb, spsum,
                                                     Exp, bias=bias,
                                                     scale=1.0)
                            else:         # lower: alibi add on DVE
                                nc.tensor.matmul(
                                    spsum,
                                    kT[:, jb * 128:(jb + 1) * 128],
                                    qT[:, i0:i0 + 512],
                                    start=True, stop=True)
                                ssb = s2w.tile([128, 512], fp16, name="ssb",
                                               tag="ssb", bufs=4)
                                nc.vector.tensor_tensor(
                                    ssb, spsum,
                                    atr[:, h * 512:(h + 1) * 512], add)
                                k_ = 4 * ib - jb
                                bias = cb[:, h * 12 + k_ - 1: h * 12 + k_]
                                nc.scalar.activation(psb, ssb,
                                                     Exp, bias=bias,
                                                     scale=1.0)
                            pend.append((h, opsum, den, jb, c0, psb))
                            if len(pend) > 3:
                                flush(pend.pop(0))
                            if hp == 2 and jb in (1, 2):
                                drain_tail()
                            # weave in proj chains for tb-1 to keep PE dense
                            avail = len(proj_jobs) - 2
                            nj = -(-avail // iters_left[0]) if avail > 0 else 0
                            for _ in range(nj):
                                emit_proj(proj_jobs.pop(0))
                            iters_left[0] -= 1
                        for ent in pend:
                            flush(ent)
                            if proj_jobs:
                                emit_proj(proj_jobs.pop(0))
                    while proj_jobs:
                        emit_proj(proj_jobs.pop(0))

                    if tb == 3:
                        # ---- final proj for the last query block ---------
                        # h0/h1 half-chains windowed across 3 PSUM pools so
                        # they overlap the h2/h3 normalization tails
                        while tails:
                            drain_tail()
                        jobs = [(t128, ob)
                                for t128 in range(12, 16)
                                for ob in range(4)]
                        ostages = {}
                        pool_cyc = (s1p_pool, po_pool, ps_pool)
                        tagc = ("s1p", "po", "ps")

                        def emit_half(job, acc2, hp_, nst):
                            t128, ob = job
                            for ti, (ov, pv_) in enumerate(
                                    ((o8v, pt8v), (olov, pt8v),
                                     (o8v, ptlov))):
                                nc.tensor.matmul(
                                    acc2,
                                    ov[:, 2 * hp_:2 * hp_ + 2,
                                       t128 * 128:t128 * 128 + 128],
                                    pv_[:, 2 * hp_:2 * hp_ + 2,
                                        ob * 512:ob * 512 + 512],
                                    start=(hp_ == 0 and ti == 0),
                                    stop=(hp_ == 1 and ti == 2),
                                    perf_mode=DR, skip_group_check=True)

                        def finish(job, acc2):
                            t128, ob = job
                            if t128 not in ostages:
                                ostages[t128] = s3o.tile(
                                    [128, 2048], bf16, name="ostage",
                                    tag="ostage")
                            ostage = ostages[t128]
                            emit_half(job, acc2, 1, False)
                            if ob < 2:
                                nc.scalar.activation(
                                    ostage[:, ob * 512:(ob + 1) * 512],
                                    acc2, Ident, scale=1.0 / SPW)
                            else:
                                nc.vector.tensor_scalar_mul(
                                    ostage[:, ob * 512:(ob + 1) * 512],
                                    acc2, 1.0 / SPW)
                            if ob == 1:
                                nc.sync.dma_start(
                                    out_d[t128 * 128:(t128 + 1) * 128,
                                          0:1024], ostage[:, 0:1024])
                            elif ob == 3:
                                nc.sync.dma_start(
                                    out_d[t128 * 128:(t128 + 1) * 128,
                                          1024:2048], ostage[:, 1024:2048])

                        win = []
                        for idx, job in enumerate(jobs):
                            acc2 = pool_cyc[idx % 3].tile(
                                [128, 512], f32, name="acc2w",
                                tag=tagc[idx % 3])
                            emit_half(job, acc2, 0, True)
                            win.append((job, acc2))
                            if len(win) >= 6:
                                jo, a = win.pop(0)
                                finish(jo, a)
                        while win:
                            jo, a = win.pop(0)
                            finish(jo, a)
                        if _dbg:
                            nc.sync.dma_start(dbgq_d, qkvT.bitcast(f32))
                            nc.sync.dma_start(dbgv_d, v_all)
                            nc.sync.dma_start(
                                dbgo_d, o8.bitcast(mybir.dt.uint8))
                            nc.sync.dma_start(
                                dbgl_d, olo.bitcast(mybir.dt.uint8))

    nc.compile()
    return nc


def get_nc():
    if "nc" not in _CACHE:
        _CACHE["nc"] = _build_nc()
    return _CACHE["nc"]


# --------------------------------------------------------------------------
# host-side packing
# --------------------------------------------------------------------------

def _expected_slopes():
    return 2.0 ** (-8.0 * (np.arange(1, NH + 1) / NH))  # float64


def _check_structure(attn_mask, alibi_bias):
    """Return exact float64 alibi slopes if inputs match the expected
    causal-mask + rank-1 alibi structure, else None."""
    am = np.asarray(attn_mask)
    if am.shape != (1, 1, T, T):
        return None
    if not np.array_equal(am[0, 0], np.tril(np.ones((T, T), dtype=bool))):
        return None
    al = np.asarray(alibi_bias, dtype=np.float32)
    if al.shape != (1, NH, T, T):
        return None
    slopes = _expected_slopes()
    if not np.allclose(al[0, :, 0, 1], slopes.astype(np.float32),
                       rtol=1e-6, atol=1e-8):
        return None
    idx = np.arange(T, dtype=np.float64)
    rel = idx[None, :] - idx[:, None]
    for h in range(NH):
        ref = (slopes[h] * rel).astype(np.float32)
        if not np.array_equal(al[0, h], ref):
            if not np.allclose(al[0, h], ref, rtol=1e-5, atol=1e-4):
                return None
    return slopes


def _f8(a):
    import ml_dtypes
    return np.ascontiguousarray(
        np.clip(a, -240.0, 240.0).astype(ml_dtypes.float8_e4m3))


def _f8_split(a):
    hi = _f8(a)
    lo = _f8(a - hi.astype(np.float32))
    return hi, lo


def _pack_core_inputs(x, qkv_w, qkv_b, proj_w, slopes):
    import ml_dtypes
    x = np.asarray(x, dtype=np.float32)
    qkv_w = np.asarray(qkv_w, dtype=np.float32)
    qkv_b = np.asarray(qkv_b, dtype=np.float32)
    proj_w = np.asarray(proj_w, dtype=np.float32)
    inv = np.float32(1.0 / math.sqrt(HD))

    x8s, xlos = [], []
    for b in range(B):
        # [128, dt, T] -> regroup to [128, tb, dt, 512]
        xt = (x[b].T.reshape(16, 128, T).transpose(1, 0, 2)
              .reshape(128, 16, 4, 512).transpose(0, 2, 1, 3)
              .reshape(128, 4 * 16 * 512))
        x8, xlo = _f8_split(xt)
        x8s.append(x8)
        xlos.append(xlo)

    per_g = []
    jj = np.arange(128, dtype=np.float64)[:, None]
    ii = np.arange(512, dtype=np.float64)[None, :]
    for g in range(KVH):
        Wq = qkv_w[512 * g:512 * (g + 1)] * inv * np.float32(SQ)
        Wk = qkv_w[D + 128 * g: D + 128 * (g + 1)] * np.float32(SKV)
        Wv = qkv_w[D + 512 + 128 * g: D + 512 + 128 * (g + 1)] * np.float32(SKV)
        Wc = np.concatenate([Wq, Wk, Wv], axis=0)          # [768, 2048]
        # -> [128 part, fb(6), dt(16), 128]
        wt = (Wc.T.reshape(16, 128, 768).transpose(1, 0, 2)   # [128, dt, 768]
              .reshape(128, 16, 6, 128).transpose(0, 2, 1, 3)
              .reshape(128, 6 * 16 * 128))
        w8, wlo = _f8_split(wt)
        bc = np.concatenate([qkv_b[512 * g:512 * (g + 1)] * inv,
                             qkv_b[D + 128 * g: D + 128 * (g + 1)],
                             qkv_b[D + 512 + 128 * g: D + 512 + 128 * (g + 1)]])
        bqp = np.ascontiguousarray(bc.reshape(FB, 128).T)  # [128, 6]

        atr = np.empty((128, 4 * 512), dtype=np.float32)
        atd = np.empty((128, 16 * 512), dtype=np.float16)
        cbp = np.empty((128, 48), dtype=np.float32)
        abt = np.zeros((2, 4 * 640), dtype=np.float32)
        for h in range(GRP):
            s = slopes[4 * g + h]
            atr[:, h * 512:(h + 1) * 512] = (s * (jj - ii)).astype(np.float32)
            base = h * 640
            abt[0, base:base + 128] = (s * np.arange(128)).astype(np.float32)
            abt[1, base:base + 128] = 1.0
            abt[0, base + 128:base + 640] = 1.0
            abt[1, base + 128:base + 640] = (
                -s * np.arange(512)).astype(np.float32)
            for dd in range(4):
                A = (s * (jj - ii + 128 * dd)).astype(np.float16)
                A[(jj + 128 * dd - ii) > 0] = np.float16(NEG16)
                atd[:, (h * 4 + dd) * 512:(h * 4 + dd + 1) * 512] = A
            for k_ in range(1, 13):
                cbp[:, h * 12 + k_ - 1] = np.float32(s * (-128.0 * k_))

        ptp = np.ascontiguousarray(
            proj_w[:, 512 * g:512 * (g + 1)].T
            .reshape(4, 128, T).transpose(1, 0, 2).reshape(128, 4 * T)
            * np.float32(SPW))
        pt8, ptlo = _f8_split(ptp)
        per_g.append({"w8": w8, "wlo": wlo, "bq": bqp, "abt": abt,
                      "atr": atr, "atd": atd, "cb": cbp, "pt8": pt8,
                      "ptlo": ptlo})

    knb = np.zeros((128, 129), dtype=ml_dtypes.bfloat16)
    knb[:, 0] = 1.0                     # ones column  [128, 1]
    knb[:, 1:129] = np.eye(128, dtype=np.float32)
    knh = np.ascontiguousarray(np.eye(128, dtype=np.float32))
    onh = np.ones((1, 128), dtype=np.float16)

    in_maps = []
    for c in range(NCORE):
        b, g = divmod(c, KVH)
        m = dict(per_g[g])
        m["x8"] = x8s[b]
        m["xlo"] = xlos[b]
        m["knb"] = knb
        m["knh"] = knh
        m["onh"] = onh
        in_maps.append(m)
    return in_maps


# --------------------------------------------------------------------------
# numpy fallback (only used if inputs don't match the expected structure)
# --------------------------------------------------------------------------

def _numpy_reference(x, attn_mask, alibi_bias, qkv_w, qkv_b, proj_w, proj_b):
    x = np.asarray(x, dtype=np.float32)
    b, t, c = x.shape
    qkv = x @ qkv_w.T + qkv_b
    q = qkv[..., :D].reshape(b, t, KVH, GRP, HD).transpose(0, 2, 3, 1, 4)
    k = qkv[..., D:D + 512].reshape(b, t, KVH, HD).transpose(0, 2, 1, 3)
    v = qkv[..., D + 512:].reshape(b, t, KVH, HD).transpose(0, 2, 1, 3)
    scale = 1.0 / math.sqrt(HD)
    att = np.einsum("bkgtd,bksd->bkgts", q, k).astype(np.float32) * scale
    att = att + np.asarray(alibi_bias).reshape(1, KVH, GRP, t, t)
    mask = np.asarray(attn_mask)[:, :, None]
    att = np.where(mask, att, -np.inf)
    att = att - att.max(axis=-1, keepdims=True)
    np.exp(att, out=att)
    att /= att.sum(axis=-1, keepdims=True)
    out = np.einsum("bkgts,bksd->bkgtd", att, v)
    out = out.transpose(0, 3, 1, 2, 4).reshape(b, t, c)
    return (out @ proj_w.T + proj_b).astype(np.float32)


# --------------------------------------------------------------------------
# entry point
# --------------------------------------------------------------------------

def kernel(x, attn_mask, alibi_bias, qkv_w, qkv_b, proj_w, proj_b):
    from concourse import bass_utils

    slopes = _check_structure(attn_mask, alibi_bias)
    if slopes is None:
        return _numpy_reference(x, attn_mask, alibi_bias, qkv_w, qkv_b,
                                proj_w, proj_b)

    nc = get_nc()
    in_maps = _pack_core_inputs(x, qkv_w, qkv_b, proj_w, slopes)
    res = bass_utils.run_bass_kernel_spmd(nc, in_maps,
                                          core_ids=list(range(NCORE)))
    proj_b = np.asarray(proj_b, dtype=np.float32)
    out = np.empty((B, T, D), dtype=np.float32)
    for b in range(B):
        acc = res.results[4 * b + 0]["out"].astype(np.float32)
        for g in range(1, KVH):
            acc = acc + res.results[4 * b + g]["out"].astype(np.float32)
        out[b] = acc + proj_b
    return out


# revision 35
# speedup vs baseline: 1.2290x; 1.0477x over previous
"""Trainium2 Bass kernel for CausalSelfAttention (GQA + alibi, B=2, T=2048,
d_model=2048, 16 q heads / 4 kv heads).

Sharding: 8 cores = (batch b in {0,1}) x (kv-group g in {0..3}).
Each core computes, for its (b, g):
  - QKV^T slice [768, T] via 3-term fp8 DoubleRow matmuls
    (w8*x8 + w8*xlo + wlo*x8; weights host-split hi/lo with per-fb scales,
    x host-split hi/lo) -- ~bf16 accuracy at 2x the bf16 MAC rate
  - causal attention for its 4 query heads (scores transposed S^T[j, i],
    f32r matmuls; alibi+mask additive tiles on DVE; exp on ACT -> bf16 P;
    softmax denominator via near-free transposed tiny matmuls
    (lhsT=P chunk, rhs=ones column -> [128,1] outputs); reciprocal in fp16;
    per-column broadcast rebuilt via 4 single-column PE transposes + one
    Pool-engine partition_broadcast)
  - normalized attention output written as fp8 hi+lo pair (o8 + olo)
  - partial output projection via 3-term fp8 DoubleRow matmuls
    (o8*pt8 + olo*pt8 + o8*ptlo), interleaved per 512-query block
Host sums the 4 partials per batch (upcast to f32) and adds proj_b.
"""

import math

import numpy as np

D = 2048
T = 2048
NH = 16
KVH = 4
HD = 128
GRP = 4
B = 2
NCORE = 8
FB = 6          # qkv feature tiles of 128 (4 q heads + k + v)
NEG16 = -30000.0
SQ = 256.0      # fp8 scale for q weight rows (pre-scaled by 1/sqrt(hd))
SKV = 32.0      # fp8 scale for k/v weight rows
SPW = 32.0      # fp8 scale for proj weights

_CACHE: dict = {}


# --------------------------------------------------------------------------
# device kernel
# --------------------------------------------------------------------------

def _build_nc():
    import concourse.mybir as mybir
    from concourse import bacc
    import concourse.tile as tile
    f32 = mybir.dt.float32
    f32r = mybir.dt.float32r
    bf16 = mybir.dt.bfloat16
    fp16 = mybir.dt.float16
    fp8 = mybir.dt.float8e4
    DR = mybir.MatmulPerfMode.DoubleRow
    Exp = mybir.ActivationFunctionType.Exp
    Ident = mybir.ActivationFunctionType.Identity
    add = mybir.AluOpType.add
    mult = mybir.AluOpType.mult
    subtract = mybir.AluOpType.subtract

    nc = bacc.Bacc("TRN2", target_bir_lowering=False, debug=False,
                   num_devices=NCORE)

    # x8/xlo: [tb][dt][512] per-tb contiguous; w8/wlo: [fb][dt][128]
    x8_d = nc.dram_tensor("x8", [128, 4 * 16 * 512], fp8,
                          kind="ExternalInput").ap()
    xlo_d = nc.dram_tensor("xlo", [128, 4 * 16 * 512], fp8,
                           kind="ExternalInput").ap()
    w8_d = nc.dram_tensor("w8", [128, 6 * 16 * 128], fp8,
                          kind="ExternalInput").ap()
    wlo_d = nc.dram_tensor("wlo", [128, 6 * 16 * 128], fp8,
                           kind="ExternalInput").ap()
    bq_d = nc.dram_tensor("bq", [128, FB], f32, kind="ExternalInput").ap()
    atr_d = nc.dram_tensor("atr", [128, 4 * 512], f32,
                           kind="ExternalInput").ap()
    atd_d = nc.dram_tensor("atd", [128, 16 * 512], fp16,
                           kind="ExternalInput").ap()
    cb_d = nc.dram_tensor("cb", [128, 48], f32, kind="ExternalInput").ap()
    abt_d = nc.dram_tensor("abt", [2, 4 * 640], f32r,
                           kind="ExternalInput").ap()
    pt8_d = nc.dram_tensor("pt8", [128, 4 * T], fp8, kind="ExternalInput").ap()
    ptlo_d = nc.dram_tensor("ptlo", [128, 4 * T], fp8,
                            kind="ExternalInput").ap()
    knb_d = nc.dram_tensor("knb", [128, 129], bf16, kind="ExternalInput").ap()
    knh_d = nc.dram_tensor("knh", [128, 128], f32r, kind="ExternalInput").ap()
    onh_d = nc.dram_tensor("onh", [1, 128], fp16, kind="ExternalInput").ap()
    out_d = nc.dram_tensor("out", [T, D], bf16, kind="ExternalOutput").ap()
    import os
    _dbg = os.environ.get("KDBG") == "1"
    if _dbg:
        dbgq_d = nc.dram_tensor("dbgq", [128, 5 * T], f32,
                                kind="ExternalOutput").ap()
        dbgv_d = nc.dram_tensor("dbgv", [128, T], bf16,
                                kind="ExternalOutput").ap()
        dbgo_d = nc.dram_tensor("dbgo", [128, 4 * T], mybir.dt.uint8,
                                kind="ExternalOutput").ap()
        dbgl_d = nc.dram_tensor("dbgl", [128, 4 * T], mybir.dt.uint8,
                                kind="ExternalOutput").ap()

    with tile.TileContext(nc) as tc:
        with tc.tile_pool(name="persist", bufs=1) as pp, \
             tc.tile_pool(name="ps", bufs=3, space="PSUM") as ps_pool, \
             tc.tile_pool(name="s1p", bufs=2, space="PSUM") as s1p_pool, \
             tc.tile_pool(name="po", bufs=2, space="PSUM") as po_pool, \
             tc.tile_pool(name="dr", bufs=1, space="PSUM") as dr_pool:

            qkvT = pp.tile([128, 5 * T], f32r, name="qkvT", tag="qkvT")
            vTb = pp.tile([128, T], bf16, name="vTb", tag="vTb")
            v_all = pp.tile([128, T], bf16, name="v_all", tag="v_all")
            o8 = pp.tile([128, 4 * T], fp8, name="o8", tag="o8")
            olo = pp.tile([128, 4 * T], fp8, name="olo", tag="olo")
            bq = pp.tile([128, FB], f32, name="bqs", tag="bqs")
            cb = pp.tile([128, 48], f32, name="cbs", tag="cbs")
            onesb = pp.tile([128, 1], bf16, name="onesb", tag="onesb")
            identb = pp.tile([128, 128], bf16, name="identb", tag="identb")
            identr = pp.tile([128, 128], f32r, name="identr", tag="identr")
            onesh = pp.tile([1, 128], fp16, name="onesh", tag="onesh")
            w8 = pp.tile([128, 6 * 2048], fp8, name="w8", tag="w8")
            wlo = pp.tile([128, 6 * 2048], fp8, name="wlo", tag="wlo")
            abt = pp.tile([2, 4 * 640], f32r, name="abt", tag="abt")
            atr = pp.tile([128, 4 * 512], f32, name="atr", tag="atr")
            atd = pp.tile([128, 16 * 512], fp16, name="atd", tag="atd")
            pt8 = pp.tile([128, 4 * T], fp8, name="pt8", tag="pt8")
            ptlo = pp.tile([128, 4 * T], fp8, name="ptlo", tag="ptlo")
            kT = qkvT[:, 4 * T:5 * T]
            o8v = o8.rearrange("p (h t) -> p h t", h=4)
            olov = olo.rearrange("p (h t) -> p h t", h=4)
            pt8v = pt8.rearrange("p (h t) -> p h t", h=4)
            ptlov = ptlo.rearrange("p (h t) -> p h t", h=4)

            with tc.tile_pool(name="s1x", bufs=2) as s1x, \
                 tc.tile_pool(name="s1xl", bufs=2) as s1xl, \
                 tc.tile_pool(name="s2w", bufs=2) as s2w, \
                 tc.tile_pool(name="s3o", bufs=2) as s3o:

                x8s = [None] * 4
                xlos = [None] * 4
                x8s[0] = s1x.tile([128, 16 * 512], fp8, name="x8t", tag="x8t")
                x8s[1] = s1x.tile([128, 16 * 512], fp8, name="x8t", tag="x8t")
                xlos[0] = s1xl.tile([128, 16 * 512], fp8, name="xlt", tag="xlt")
                xlos[1] = s1xl.tile([128, 16 * 512], fp8, name="xlt", tag="xlt")

                # ---- DMA issue order: minimal working set first ----------
                nc.sync.dma_start(w8[:, 0:2048], w8_d[:, 0:2048])
                nc.sync.dma_start(x8s[0][:, 0:4096], x8_d[:, 0:4096])
                nc.sync.dma_start(x8s[0][:, 4096:8192], x8_d[:, 4096:8192])
                nc.sync.dma_start(wlo[:, 0:2048], wlo_d[:, 0:2048])
                nc.sync.dma_start(bq, bq_d)
                nc.sync.dma_start(xlos[0][:, 0:4096], xlo_d[:, 0:4096])
                nc.sync.dma_start(xlos[0][:, 4096:8192], xlo_d[:, 4096:8192])
                for fb_ in range(1, 6):
                    lo = fb_ * 2048
                    nc.sync.dma_start(w8[:, lo:lo + 2048],
                                      w8_d[:, lo:lo + 2048])
                    nc.sync.dma_start(wlo[:, lo:lo + 2048],
                                      wlo_d[:, lo:lo + 2048])
                nc.sync.dma_start(cb, cb_d)
                nc.sync.dma_start(abt, abt_d)
                nc.sync.dma_start(onesb, knb_d[:, 0:1])
                nc.sync.dma_start(identb, knb_d[:, 1:129])
                nc.sync.dma_start(identr, knh_d)
                nc.sync.dma_start(onesh, onh_d)
                nc.sync.dma_start(atr, atr_d)
                nc.sync.dma_start(x8s[1], x8_d[:, 8192:16384])
                nc.sync.dma_start(xlos[1], xlo_d[:, 8192:16384])
                nc.sync.dma_start(atd, atd_d)
                nc.sync.dma_start(pt8, pt8_d)
                nc.sync.dma_start(ptlo, ptlo_d)

                tails = []  # deferred per-(h,ib) softmax-normalization tails

                def drain_tail():
                    if tails:
                        tails.pop(0)()

                for tb in range(4):
                    x8t, xlot = x8s[tb], xlos[tb]
                    if tb + 2 < 4:
                        x8s[tb + 2] = s1x.tile([128, 16 * 512], fp8,
                                               name="x8t", tag="x8t")
                        xlos[tb + 2] = s1xl.tile([128, 16 * 512], fp8,
                                                 name="xlt", tag="xlt")
                        nc.sync.dma_start(
                            x8s[tb + 2],
                            x8_d[:, (tb + 2) * 8192:(tb + 3) * 8192])
                        nc.sync.dma_start(
                            xlos[tb + 2],
                            xlo_d[:, (tb + 2) * 8192:(tb + 3) * 8192])
                    x8v = x8t.rearrange("p (d n) -> p d n", d=16)
                    xlov = xlot.rearrange("p (d n) -> p d n", d=16)
                    w8v = w8.rearrange("p (f d m) -> p f d m", f=6, d=16)
                    wlov = wlo.rearrange("p (f d m) -> p f d m", f=6, d=16)

                    # ---- stage 1: per-fb 3-term fp8 DoubleRow chains -----
                    for fb in range(FB):
                        acc = s1p_pool.tile([128, 512], f32, name="acc",
                                            tag="s1p")
                        nmm = 0
                        for wv, xv in ((w8v, x8v), (wlov, x8v), (w8v, xlov)):
                            for dp in range(8):
                                nc.tensor.matmul(
                                    acc,
                                    wv[:, fb, 2 * dp:2 * dp + 2, :],
                                    xv[:, 2 * dp:2 * dp + 2, :],
                                    start=(nmm == 0), stop=(nmm == 23),
                                    perf_mode=DR)
                                nmm += 1
                        if fb < 4:
                            dst = qkvT[:, fb * T + tb * 512:
                                           fb * T + tb * 512 + 512]
                            sc = 1.0 / SQ
                        elif fb == 4:
                            dst = kT[:, tb * 512:tb * 512 + 512]
                            sc = 1.0 / SKV
                        else:
                            dst = vTb[:, tb * 512:tb * 512 + 512]
                            sc = 1.0 / SKV
                        nc.scalar.activation(dst, acc, Ident,
                                             bias=bq[:, fb:fb + 1],
                                             scale=sc)
                        if fb in (2, 4):
                            drain_tail()

                    # ---- V transposes for this tb's four 128-blocks ------
                    for jt in range(4 * tb, 4 * tb + 4):
                        pv = ps_pool.tile([128, 128], bf16, name="pv",
                                          tag="ps")
                        nc.tensor.transpose(
                            pv, vTb[:, jt * 128:(jt + 1) * 128], identb)
                        nc.vector.tensor_copy(
                            v_all[:, jt * 128:(jt + 1) * 128], pv)

                    # ---- proj jobs for tb-1, woven into attention below --
                    proj_jobs = []
                    if tb >= 1:
                        for t128 in range(4 * (tb - 1), 4 * (tb - 1) + 4):
                            for ob in range(4):
                                proj_jobs.append((t128, ob))
                    ostages = {}

                    def emit_proj(job):
                        t128, ob = job
                        if t128 not in ostages:
                            ostages[t128] = s3o.tile([128, 2048], bf16,
                                                     name="ostage",
                                                     tag="ostage")
                        ostage = ostages[t128]
                        acc2 = s1p_pool.tile([128, 512], f32, name="acc2",
                                             tag="s1p")
                        nmm = 0
                        for hp_ in range(2):
                            for ov, pv_ in ((o8v, pt8v), (olov, pt8v),
                                            (o8v, ptlov)):
                                nc.tensor.matmul(
                                    acc2,
                                    ov[:, 2 * hp_:2 * hp_ + 2,
                                       t128 * 128:t128 * 128 + 128],
                                    pv_[:, 2 * hp_:2 * hp_ + 2,
                                        ob * 512:ob * 512 + 512],
                                    start=(nmm == 0), stop=(nmm == 5),
                                    perf_mode=DR)
                                nmm += 1
                        if ob < 2:
                            nc.scalar.activation(
                                ostage[:, ob * 512:(ob + 1) * 512], acc2,
                                Ident, scale=1.0 / SPW)
                        else:
                            nc.vector.tensor_scalar_mul(
                                ostage[:, ob * 512:(ob + 1) * 512], acc2,
                                1.0 / SPW)
                        if ob == 1:
                            nc.sync.dma_start(
                                out_d[t128 * 128:(t128 + 1) * 128, 0:1024],
                                ostage[:, 0:1024])
                        elif ob == 3:
                            nc.sync.dma_start(
                                out_d[t128 * 128:(t128 + 1) * 128,
                                      1024:2048],
                                ostage[:, 1024:2048])

                    # ---- attention at ib = tb, head pairs interleaved ----
                    ib = tb
                    njb = 4 * (ib + 1)
                    iters_left = [4 * njb]
                    for hp in (0, 2):
                        denb = dr_pool.tile([128, 8], f32, name="denb",
                                            tag="dr")
                        chains = []
                        for idx, h in enumerate((hp, hp + 1)):
                            opsum = po_pool.tile([128, 512], f32,
                                                 name="opsum", tag="po")
                            den = denb[:, 4 * idx:4 * idx + 4]
                            # only the very first den matmul of the PAIR may
                            # start=True: PSUM start marks the whole 2KB bank
                            # pending-zero, so chain 2 and chunks 1-3 rely on
                            # the pending bytes from that single start.
                            chains.append((h, opsum, den, idx == 0))

                        def flush(ent):
                            (h, opsum, den, first, jb, c0, psb) = ent
                            for c in range(c0 // 128, 4):
                                nc.tensor.matmul(
                                    den[:, c:c + 1],
                                    psb[:, c * 128:(c + 1) * 128],
                                    onesb,
                                    start=(jb == 0 and c == 0 and first),
                                    stop=(jb == 4 * ib + c),
                                    skip_group_check=True)
                            nc.tensor.matmul(
                                opsum[:, c0:512],
                                v_all[:, jb * 128:(jb + 1) * 128],
                                psb[:, c0:512],
                                start=(jb == 0), stop=(jb == njb - 1),
                                skip_group_check=True)
                            if jb == njb - 1:
                                rden = s2w.tile([128, 4], f32r,
                                                name="rden", tag="rden",
                                                bufs=4)
                                with nc.allow_low_precision(
                                        reason="softmax recip in f32r"):
                                    nc.vector.reciprocal(rden, den)
                                # snapshot opsum so the PSUM bank frees now
                                osb = s2w.tile([128, 512], fp16,
                                               name="osb", tag="osb",
                                               bufs=4)
                                nc.vector.tensor_copy(osb, opsum)

                                def tail(h=h, ib=ib, rden=rden, osb=osb):
                                    rps = ps_pool.tile([128, 512], f32,
                                                       name="rps", tag="ps")
                                    rrow = rps[0:1, 0:512].bitcast(f32r)
                                    for c in range(4):
                                        nc.tensor.matmul(
                                            rrow[:, c * 128:(c + 1) * 128],
                                            rden[:, c:c + 1], identr,
                                            is_transpose=True,
                                            start=True, stop=True,
                                            skip_group_check=True)
                                    rsum = s2w.tile([1, 512], fp16,
                                                    name="rsum", tag="rsum")
                                    nc.scalar.copy(rsum, rps[0:1, 0:512])
                                    nc.tensor.matmul(rps, onesh, rsum,
                                                     start=True, stop=True,
                                                     skip_group_check=True)
                                    tmp = s2w.tile([128, 512], f32,
                                                   name="otmp", tag="otmp")
                                    nc.vector.tensor_tensor(tmp, osb, rps,
                                                            mult)
                                    od = o8v[:, h, ib * 512:ib * 512 + 512]
                                    ol = olov[:, h, ib * 512:ib * 512 + 512]
                                    nc.vector.tensor_copy(od, tmp)
                                    nc.vector.tensor_tensor(
                                        ol, tmp, od, subtract)
                                tails.append(tail)

                        pend = []
                        for jb in range(njb):
                          for (h, opsum, den, first) in chains:
                            qT = qkvT[:, h * T:(h + 1) * T]
                            i0 = ib * 512
                            dd = jb - 4 * ib
                            c0 = 128 * dd if dd > 0 else 0
                            m0 = min(c0, 256)
                            spsum = ps_pool.tile([128, 512], f32,
                                                 name="spsum", tag="ps")
                            psb = s2w.tile([128, 512], bf16, name="psb",
                                           tag="psb", bufs=8)
                            if dd >= 0:   # diagonal band (masked fp16)
                                nc.tensor.matmul(
                                    spsum[:, m0:512],
                                    kT[:, jb * 128:(jb + 1) * 128],
                                    qT[:, i0 + m0:i0 + 512],
                                    start=True, stop=True)
                                ssb = s2w.tile([128, 512], fp16, name="ssb",
                                               tag="ssb", bufs=4)
                                nc.vector.tensor_tensor(
                                    ssb[:, c0:512], spsum[:, c0:512],
                                    atd[:, (h * 4 + dd) * 512 + c0:
                                          (h * 4 + dd + 1) * 512], add)
                                nc.scalar.activation(psb[:, c0:512],
                                                     ssb[:, c0:512],
                                                     Exp, bias=0.0, scale=1.0)
                            elif jb % 2 == 0:  # lower: rank-2 alibi on PE
                                nc.tensor.matmul(
                                    spsum,
                                    kT[:, jb * 128:(jb + 1) * 128],
                                    qT[:, i0:i0 + 512],
                                    start=True, stop=False,
                                    skip_group_check=True)
                                nc.tensor.matmul(
                                    spsum,
                                    abt[:, h * 640:h * 640 + 128],
                                    abt[:, h * 640 + 128:(h + 1) * 640],
                                    start=False, stop=True,
                                    skip_group_check=True)
                                k_ = 4 * ib - jb
                                bias = cb[:, h * 12 + k_ - 1: h * 12 + k_]
                                nc.scalar.activation(psb, spsum,
                                                     Exp, bias=bias,
                                                     scale=1.0)
                            else:         # lower: alibi add on DVE
                                nc.tensor.matmul(
                                    spsum,
                                    kT[:, jb * 128:(jb + 1) * 128],
                                    qT[:, i0:i0 + 512],
                                    start=True, stop=True)
                                ssb = s2w.tile([128, 512], fp16, name="ssb",
                                               tag="ssb", bufs=4)
                                nc.vector.tensor_tensor(
                                    ssb, spsum,
                                    atr[:, h * 512:(h + 1) * 512], add)
                                k_ = 4 * ib - jb
                                bias = cb[:, h * 12 + k_ - 1: h * 12 + k_]
                                nc.scalar.activation(psb, ssb,
                                                     Exp, bias=bias,
                                                     scale=1.0)
                            pend.append((h, opsum, den, first, jb, c0,
                                         psb))
                            if len(pend) > 3:
                                flush(pend.pop(0))
                            if hp == 2 and jb in (1, 2):
                                drain_tail()
                            # weave in proj chains for tb-1 to keep PE dense
                            avail = len(proj_jobs) - 2
                            nj = -(-avail // iters_left[0]) if avail > 0 else 0
                            for _ in range(nj):
                                emit_proj(proj_jobs.pop(0))
                            iters_left[0] -= 1
                        for ent in pend:
                            flush(ent)
                            if proj_jobs:
                                emit_proj(proj_jobs.pop(0))
                    while proj_jobs:
                        emit_proj(proj_jobs.pop(0))

                    if tb == 3:
                        # ---- final proj for the last query block ---------
                        # h0/h1 half-chains windowed across 3 PSUM pools so
                        # they overlap the h2/h3 normalization tails
                        while tails:
                            drain_tail()
                        jobs = [(t128, ob)
                                for t128 in range(12, 16)
                                for ob in range(4)]
                        ostages = {}
                        pool_cyc = (s1p_pool, po_pool, ps_pool)
                        tagc = ("s1p", "po", "ps")

                        def emit_half(job, acc2, hp_, nst):
                            t128, ob = job
                            for ti, (ov, pv_) in enumerate(
                                    ((o8v, pt8v), (olov, pt8v),
                                     (o8v, ptlov))):
                                nc.tensor.matmul(
                                    acc2,
                                    ov[:, 2 * hp_:2 * hp_ + 2,
                                       t128 * 128:t128 * 128 + 128],
                                    pv_[:, 2 * hp_:2 * hp_ + 2,
                                        ob * 512:ob * 512 + 512],
                                    start=(hp_ == 0 and ti == 0),
                                    stop=(hp_ == 1 and ti == 2),
                                    perf_mode=DR, skip_group_check=True)

                        def finish(job, acc2):
                            t128, ob = job
                            if t128 not in ostages:
                                ostages[t128] = s3o.tile(
                                    [128, 2048], bf16, name="ostage",
                                    tag="ostage")
                            ostage = ostages[t128]
                            emit_half(job, acc2, 1, False)
                            if ob < 2:
                                nc.scalar.activation(
                                    ostage[:, ob * 512:(ob + 1) * 512],
                                    acc2, Ident, scale=1.0 / SPW)
                            else:
                                nc.vector.tensor_scalar_mul(
                                    ostage[:, ob * 512:(ob + 1) * 512],
                                    acc2, 1.0 / SPW)
                            if ob == 1:
                                nc.sync.dma_start(
                                    out_d[t128 * 128:(t128 + 1) * 128,
                                          0:1024], ostage[:, 0:1024])
                            elif ob == 3:
                                nc.sync.dma_start(
                                    out_d[t128 * 128:(t128 + 1) * 128,
                                          1024:2048], ostage[:, 1024:2048])

                        win = []
                        for idx, job in enumerate(jobs):
                            acc2 = pool_cyc[idx % 3].tile(
                                [128, 512], f32, name="acc2w",
                                tag=tagc[idx % 3])
                            emit_half(job, acc2, 0, True)
                            win.append((job, acc2))
                            if len(win) >= 6:
                                jo, a = win.pop(0)
                                finish(jo, a)
                        while win:
                            jo, a = win.pop(0)
                            finish(jo, a)
                        if _dbg:
                            nc.sync.dma_start(dbgq_d, qkvT.bitcast(f32))
                            nc.sync.dma_start(dbgv_d, v_all)
                            nc.sync.dma_start(
                                dbgo_d, o8.bitcast(mybir.dt.uint8))
                            nc.sync.dma_start(
                                dbgl_d, olo.bitcast(mybir.dt.uint8))

    nc.compile()
    return nc


def get_nc():
    if "nc" not in _CACHE:
        _CACHE["nc"] = _build_nc()
    return _CACHE["nc"]


# --------------------------------------------------------------------------
# host-side packing
# --------------------------------------------------------------------------

def _expected_slopes():
    return 2.0 ** (-8.0 * (np.arange(1, NH + 1) / NH))  # float64


def _check_structure(attn_mask, alibi_bias):
    """Return exact float64 alibi slopes if inputs match the expected
    causal-mask + rank-1 alibi structure, else None."""
    am = np.asarray(attn_mask)
    if am.shape != (1, 1, T, T):
        return None
    if not np.array_equal(am[0, 0], np.tril(np.ones((T, T), dtype=bool))):
        return None
    al = np.asarray(alibi_bias, dtype=np.float32)
    if al.shape != (1, NH, T, T):
        return None
    slopes = _expected_slopes()
    if not np.allclose(al[0, :, 0, 1], slopes.astype(np.float32),
                       rtol=1e-6, atol=1e-8):
        return None
    idx = np.arange(T, dtype=np.float64)
    rel = idx[None, :] - idx[:, None]
    for h in range(NH):
        ref = (slopes[h] * rel).astype(np.float32)
        if not np.array_equal(al[0, h], ref):
            if not np.allclose(al[0, h], ref, rtol=1e-5, atol=1e-4):
                return None
    return slopes


def _f8(a):
    import ml_dtypes
    return np.ascontiguousarray(
        np.clip(a, -240.0, 240.0).astype(ml_dtypes.float8_e4m3))


def _f8_split(a):
    hi = _f8(a)
    lo = _f8(a - hi.astype(np.float32))
    return hi, lo


def _pack_core_inputs(x, qkv_w, qkv_b, proj_w, slopes):
    import ml_dtypes
    x = np.asarray(x, dtype=np.float32)
    qkv_w = np.asarray(qkv_w, dtype=np.float32)
    qkv_b = np.asarray(qkv_b, dtype=np.float32)
    proj_w = np.asarray(proj_w, dtype=np.float32)
    inv = np.float32(1.0 / math.sqrt(HD))

    x8s, xlos = [], []
    for b in range(B):
        # [128, dt, T] -> regroup to [128, tb, dt, 512]
        xt = (x[b].T.reshape(16, 128, T).transpose(1, 0, 2)
              .reshape(128, 16, 4, 512).transpose(0, 2, 1, 3)
              .reshape(128, 4 * 16 * 512))
        x8, xlo = _f8_split(xt)
        x8s.append(x8)
        xlos.append(xlo)

    per_g = []
    jj = np.arange(128, dtype=np.float64)[:, None]
    ii = np.arange(512, dtype=np.float64)[None, :]
    for g in range(KVH):
        Wq = qkv_w[512 * g:512 * (g + 1)] * inv * np.float32(SQ)
        Wk = qkv_w[D + 128 * g: D + 128 * (g + 1)] * np.float32(SKV)
        Wv = qkv_w[D + 512 + 128 * g: D + 512 + 128 * (g + 1)] * np.float32(SKV)
        Wc = np.concatenate([Wq, Wk, Wv], axis=0)          # [768, 2048]
        # -> [128 part, fb(6), dt(16), 128]
        wt = (Wc.T.reshape(16, 128, 768).transpose(1, 0, 2)   # [128, dt, 768]
              .reshape(128, 16, 6, 128).transpose(0, 2, 1, 3)
              .reshape(128, 6 * 16 * 128))
        w8, wlo = _f8_split(wt)
        bc = np.concatenate([qkv_b[512 * g:512 * (g + 1)] * inv,
                             qkv_b[D + 128 * g: D + 128 * (g + 1)],
                             qkv_b[D + 512 + 128 * g: D + 512 + 128 * (g + 1)]])
        bqp = np.ascontiguousarray(bc.reshape(FB, 128).T)  # [128, 6]

        atr = np.empty((128, 4 * 512), dtype=np.float32)
        atd = np.empty((128, 16 * 512), dtype=np.float16)
        cbp = np.empty((128, 48), dtype=np.float32)
        abt = np.zeros((2, 4 * 640), dtype=np.float32)
        for h in range(GRP):
            s = slopes[4 * g + h]
            atr[:, h * 512:(h + 1) * 512] = (s * (jj - ii)).astype(np.float32)
            base = h * 640
            abt[0, base:base + 128] = (s * np.arange(128)).astype(np.float32)
            abt[1, base:base + 128] = 1.0
            abt[0, base + 128:base + 640] = 1.0
            abt[1, base + 128:base + 640] = (
                -s * np.arange(512)).astype(np.float32)
            for dd in range(4):
                A = (s * (jj - ii + 128 * dd)).astype(np.float16)
                A[(jj + 128 * dd - ii) > 0] = np.float16(NEG16)
                atd[:, (h * 4 + dd) * 512:(h * 4 + dd + 1) * 512] = A
            for k_ in range(1, 13):
                cbp[:, h * 12 + k_ - 1] = np.float32(s * (-128.0 * k_))

        ptp = np.ascontiguousarray(
            proj_w[:, 512 * g:512 * (g + 1)].T
            .reshape(4, 128, T).transpose(1, 0, 2).reshape(128, 4 * T)
            * np.float32(SPW))
        pt8, ptlo = _f8_split(ptp)
        per_g.append({"w8": w8, "wlo": wlo, "bq": bqp, "abt": abt,
                      "atr": atr, "atd": atd, "cb": cbp, "pt8": pt8,
                      "ptlo": ptlo})

    knb = np.zeros((128, 129), dtype=ml_dtypes.bfloat16)
    knb[:, 0] = 1.0                     # ones column  [128, 1]
    knb[:, 1:129] = np.eye(128, dtype=np.float32)
    knh = np.ascontiguousarray(np.eye(128, dtype=np.float32))
    onh = np.ones((1, 128), dtype=np.float16)

    in_maps = []
    for c in range(NCORE):
        b, g = divmod(c, KVH)
        m = dict(per_g[g])
        m["x8"] = x8s[b]
        m["xlo"] = xlos[b]
        m["knb"] = knb
        m["knh"] = knh
        m["onh"] = onh
        in_maps.append(m)
    return in_maps


# --------------------------------------------------------------------------
# numpy fallback (only used if inputs don't match the expected structure)
# --------------------------------------------------------------------------

def _numpy_reference(x, attn_mask, alibi_bias, qkv_w, qkv_b, proj_w, proj_b):
    x = np.asarray(x, dtype=np.float32)
    b, t, c = x.shape
    qkv = x @ qkv_w.T + qkv_b
    q = qkv[..., :D].reshape(b, t, KVH, GRP, HD).transpose(0, 2, 3, 1, 4)
    k = qkv[..., D:D + 512].reshape(b, t, KVH, HD).transpose(0, 2, 1, 3)
    v = qkv[..., D + 512:].reshape(b, t, KVH, HD).transpose(0, 2, 1, 3)
    scale = 1.0 / math.sqrt(HD)
    att = np.einsum("bkgtd,bksd->bkgts", q, k).astype(np.float32) * scale
    att = att + np.asarray(alibi_bias).reshape(1, KVH, GRP, t, t)
    mask = np.asarray(attn_mask)[:, :, None]
    att = np.where(mask, att, -np.inf)
    att = att - att.max(axis=-1, keepdims=True)
    np.exp(att, out=att)
    att /= att.sum(axis=-1, keepdims=True)
    out = np.einsum("bkgts,bksd->bkgtd", att, v)
    out = out.transpose(0, 3, 1, 2, 4).reshape(b, t, c)
    return (out @ proj_w.T + proj_b).astype(np.float32)


# --------------------------------------------------------------------------
# entry point
# --------------------------------------------------------------------------

def kernel(x, attn_mask, alibi_bias, qkv_w, qkv_b, proj_w, proj_b):
    from concourse import bass_utils

    slopes = _check_structure(attn_mask, alibi_bias)
    if slopes is None:
        return _numpy_reference(x, attn_mask, alibi_bias, qkv_w, qkv_b,
                                proj_w, proj_b)

    nc = get_nc()
    in_maps = _pack_core_inputs(x, qkv_w, qkv_b, proj_w, slopes)
    res = bass_utils.run_bass_kernel_spmd(nc, in_maps,
                                          core_ids=list(range(NCORE)))
    proj_b = np.asarray(proj_b, dtype=np.float32)
    out = np.empty((B, T, D), dtype=np.float32)
    for b in range(B):
        acc = res.results[4 * b + 0]["out"].astype(np.float32)
        for g in range(1, KVH):
            acc = acc + res.results[4 * b + g]["out"].astype(np.float32)
        out[b] = acc + proj_b
    return out
